# revision 5
# baseline (speedup 1.0000x reference)
"""Trainium2 Bass kernel: batched single-head causal attention.

Problem: x [8, 2048, 1024] f32; Wq/Wk/Wv [64, 1024] f32.
  Q = x @ Wq.T; K = x @ Wk.T; V = x @ Wv.T            (per batch)
  out = softmax(mask(Q K^T / sqrt(1024))) @ V          -> [8, 2048, 64]

Sharding: data-parallel over batch B=8 across the 8 NeuronCores (one batch
element per core); the small weights are replicated.

Per-core algorithm (T=2048, C=1024, H=64, all fp32 data):
  - x is transposed on-device via PE transposes into xT [C, T] (C-chunked on
    partitions), since every projection contracts over C.
  - Projections computed in transposed layout: QT [64, T] (scaled by 1/32,
    folded into Wq on host) and a stacked KVT [128, T] (KT rows 0:64,
    VT rows 64:128).  Stacking K with V keeps KT at partition base 0 (both
    scores operands must sit in the same partition range), while VT at base
    64 is fine because it is only consumed by PE transposes.
  - V is re-transposed to natural layout V_aug [T, 65] with a ones column
    appended; the ones column makes the attention row-sums fall out of the
    same matmul that computes attn @ V.
  - Scores are computed TRANSPOSED: sT[tk, tq] = K Q^T blocks [128, 512];
    causality skips all fully-masked blocks; softmax max-subtraction is
    skipped entirely (scores are provably tiny: |s| <~ 1.5) so softmax is
    just exp + normalize; exp runs on ACT straight out of PSUM; diagonal
    blocks are multiplied by precomputed 0/1 masks (exp of masked entries
    becomes exactly 0, matching exp(-inf)).
  - outT_aug [65, 512] = V_aug^T @ expT accumulated over tk chunks: rows
    0:64 unnormalized out^T, row 64 = row-sums.  A final PE transpose gives
    natural [128, 65] chunks; DVE reciprocal + tensor_scalar multiply
    normalizes; DMA out.

Matmuls run in float32r (1 row/cycle vs 4 for float32).  The BIR verifier
requires every SBUF tensor consumed by an FP32r matmul to be produced by an
instruction with f32r output ("rounded").  All matmul operands here are
produced by ACT/DVE copies out of PSUM anyway, so those copies declare f32r
output for free; the only extra work is one rounding copy for the weights.
"""

import numpy as np

import concourse.bass as bass
import concourse.mybir as mybir
import concourse.tile as tile
from concourse import bacc
from concourse.bass_utils import run_bass_kernel_spmd

B = 8
T = 2048
C = 1024
H = 64
P = 128
NT = T // P   # 16 row chunks
NCH = C // P  # 8 contraction chunks
NB = 4        # tq blocks
BQ = 512      # tq block size
F32 = mybir.dt.float32
F32R = mybir.dt.float32r

# float32r matmuls: 1 row/cycle (vs 4 for float32) when moving free >= 256.
MM_FP32R = True


def _mm(ap):
    """Matmul-operand view."""
    return ap.bitcast(F32R) if MM_FP32R else ap


def _rnd(ap):
    """Producer-output view for tensors later consumed by f32r matmuls."""
    return ap.bitcast(F32R) if MM_FP32R else ap


def build_nc():
    nc = bacc.Bacc("TRN2", target_bir_lowering=False)
    x_d = nc.dram_tensor("x", [T, C], F32, kind="ExternalInput").ap()
    w_d = nc.dram_tensor("w", [C, 192], F32, kind="ExternalInput").ap()
    m_d = nc.dram_tensor("masks", [P, 4, BQ], F32, kind="ExternalInput").ap()
    i_d = nc.dram_tensor("ident", [P, P], F32, kind="ExternalInput").ap()
    o_d = nc.dram_tensor("out", [T, H], F32, kind="ExternalOutput").ap()

    with tile.TileContext(nc) as tc:
        _emit(nc, tc, x_d, w_d, m_d, i_d, o_d)
    nc.compile()
    return nc


def _emit(nc, tc, x_d, w_d, m_d, i_d, o_d):
    import contextlib

    ctx = contextlib.ExitStack()
    with ctx:
        consts = ctx.enter_context(tc.tile_pool(name="consts", bufs=1))
        xnat = ctx.enter_context(tc.tile_pool(name="xnat", bufs=4))
        persist = ctx.enter_context(tc.tile_pool(name="persist", bufs=1))
        expp = ctx.enter_context(tc.tile_pool(name="expp", bufs=4))
        oaugp = ctx.enter_context(tc.tile_pool(name="oaugp", bufs=2))
        outp = ctx.enter_context(tc.tile_pool(name="outp", bufs=4))
        recp = ctx.enter_context(tc.tile_pool(name="recp", bufs=4))
        # PSUM: psT 2 + psP 3 + psC 3 = 8 banks exactly
        psT = ctx.enter_context(tc.tile_pool(name="psT", bufs=2, space="PSUM"))
        psP = ctx.enter_context(tc.tile_pool(name="psP", bufs=3, space="PSUM"))
        psC = ctx.enter_context(tc.tile_pool(name="psC", bufs=3, space="PSUM"))

        # ---- constants ----
        w_raw = consts.tile([P, NCH, 192], F32, tag="w_raw")
        nc.sync.dma_start(out=w_raw, in_=w_d.rearrange("(k p) m -> p k m", p=P))
        w_sb = consts.tile([P, NCH, 192], F32, tag="w")
        nc.vector.tensor_copy(out=_rnd(w_sb), in_=w_raw)  # f32r rounding copy
        masks_sb = consts.tile([P, 4, BQ], F32, tag="masks")
        nc.sync.dma_start(out=masks_sb, in_=m_d)
        ident_sb = consts.tile([P, P], F32, tag="ident")
        nc.sync.dma_start(out=ident_sb, in_=i_d)

        # ---- persistent tiles ----
        xT = persist.tile([P, NCH, T], F32, tag="xT")      # xT[p,k,t] = x[t, k*128+p]
        qt = persist.tile([64, T], F32, tag="qt")          # QT (pre-scaled by 1/32)
        kvt = persist.tile([P, T], F32, tag="kvt")         # rows 0:64 KT, 64:128 VT
        vaug = persist.tile([P, NT, H + 1], F32, tag="vaug")  # V chunks + ones col

        ones_sb = consts.tile([P, NT, 1], F32, tag="ones")
        nc.vector.memset(ones_sb, 1.0)
        nc.vector.tensor_copy(out=_rnd(vaug[:, :, H : H + 1]), in_=ones_sb)

        ncopy = 0  # alternate ACT / DVE for PSUM->SBUF copies

        def copy(out, in_):
            nonlocal ncopy
            if ncopy % 2 == 0:
                nc.scalar.copy(out=out, in_=in_)
            else:
                nc.vector.tensor_copy(out=out, in_=in_)
            ncopy += 1

        # lag-2 pipeline of chunk-wise score -> exp/mask -> AV matmul
        pending = []

        def flush_av(limit):
            while len(pending) > limit:
                av_t, ex_t, i_, last_ = pending.pop(0)
                nc.tensor.matmul(
                    av_t[0:65, :],
                    lhsT=_mm(vaug[:, i_, 0 : H + 1]),
                    rhs=_mm(ex_t),
                    start=(i_ == 0),
                    stop=last_,
                )

        for n in range(NB):
            # ---- stage A: load x rows, PE-transpose into xT ----
            for j in range(4 * n, 4 * n + 4):
                xt = xnat.tile([P, C], F32, tag="xt")
                nc.sync.dma_start(out=xt, in_=x_d[j * P : (j + 1) * P, :])
                for half in range(2):
                    ps = psT.tile([P, 4, P], F32, tag="pst")
                    for kk in range(4):
                        k = half * 4 + kk
                        nc.tensor.transpose(
                            out=ps[:, kk, :],
                            in_=xt[:, k * P : (k + 1) * P],
                            identity=ident_sb,
                        )
                    copy(
                        _rnd(xT[:, half * 4 : half * 4 + 4, j * P : (j + 1) * P]),
                        ps,
                    )

            # ---- stage B: projections for tq block n ----
            q_ps = psP.tile([64, BQ], F32, tag="psp")
            for k in range(NCH):
                nc.tensor.matmul(
                    q_ps,
                    lhsT=_mm(w_sb[:, k, 0:64]),
                    rhs=_mm(xT[:, k, n * BQ : (n + 1) * BQ]),
                    start=(k == 0),
                    stop=(k == NCH - 1),
                )
            copy(_rnd(qt[:, n * BQ : (n + 1) * BQ]), q_ps)
            kv_ps = psP.tile([P, BQ], F32, tag="psp")
            for k in range(NCH):
                nc.tensor.matmul(
                    kv_ps,
                    lhsT=_mm(w_sb[:, k, 64:192]),
                    rhs=_mm(xT[:, k, n * BQ : (n + 1) * BQ]),
                    start=(k == 0),
                    stop=(k == NCH - 1),
                )
            copy(_rnd(kvt[:, n * BQ : (n + 1) * BQ]), kv_ps)

            # ---- stage B5: V natural chunks for this block ----
            for j in range(4 * n, 4 * n + 4):
                vp = psT.tile([P, H], F32, tag="pst")
                nc.tensor.transpose(
                    out=vp,
                    in_=kvt[64:128, j * P : (j + 1) * P],
                    identity=ident_sb[64:128, 64:128],
                )
                copy(_rnd(vaug[:, j, 0:H]), vp)

            # ---- stage C: scores^T, exp, mask, AV accumulation ----
            nchunks = 4 * (n + 1)
            av = psP.tile([65, BQ], F32, tag="psp")
            for i in range(nchunks):
                sp = psC.tile([P, BQ], F32, tag="psc")
                nc.tensor.matmul(
                    sp,
                    lhsT=_mm(kvt[0:64, i * P : (i + 1) * P]),
                    rhs=_mm(qt[:, n * BQ : (n + 1) * BQ]),
                    start=True,
                    stop=True,
                )
                ex = expp.tile([P, BQ], F32, tag="ex")
                nc.scalar.activation(
                    out=_rnd(ex), in_=sp, func=mybir.ActivationFunctionType.Exp
                )
                if i >= 4 * n:
                    nc.vector.tensor_mul(_rnd(ex), ex, masks_sb[:, i - 4 * n, :])
                pending.append((av, ex, i, i == nchunks - 1))
                flush_av(2)
            flush_av(0)

            # ---- stage D: transpose back, normalize, store ----
            oa = oaugp.tile([65, BQ], F32, tag="oa")
            copy(_rnd(oa), av)
            for q in range(4):
                tq = n * 4 + q
                tp = psC.tile([P, 72], F32, tag="psc")
                nc.tensor.transpose(
                    out=tp[:, 0:65],
                    in_=oa[:, q * P : (q + 1) * P],
                    identity=ident_sb[0:65, 0:65],
                )
                r = recp.tile([P, 1], F32, tag="r")
                nc.vector.reciprocal(r, tp[:, 64:65])
                ot = outp.tile([P, H], F32, tag="ot")
                nc.vector.tensor_scalar_mul(ot, tp[:, 0:64], r)
                nc.sync.dma_start(out=o_d[tq * P : (tq + 1) * P, :], in_=ot)


def host_inputs(Wq, Wk, Wv):
    """Replicated per-core constant inputs from the raw weights."""
    scale = np.float32(1.0 / np.sqrt(np.float32(C)))
    w = np.empty((C, 192), dtype=np.float32)
    w[:, 0:64] = Wq.T * scale
    w[:, 64:128] = Wk.T
    w[:, 128:192] = Wv.T
    p = np.arange(P, dtype=np.int64)[:, None, None]
    d = np.arange(4, dtype=np.int64)[None, :, None]
    j = np.arange(BQ, dtype=np.int64)[None, None, :]
    masks = ((p + 128 * d) <= j).astype(np.float32)
    ident = np.eye(P, dtype=np.float32)
    return w, masks, ident


def kernel(x, Wq, Wk, Wv):
    x = np.ascontiguousarray(np.asarray(x, dtype=np.float32))
    Wq = np.asarray(Wq, dtype=np.float32)
    Wk = np.asarray(Wk, dtype=np.float32)
    Wv = np.asarray(Wv, dtype=np.float32)
    assert x.shape == (B, T, C), x.shape

    w, masks, ident = host_inputs(Wq, Wk, Wv)
    nc = build_nc()
    in_maps = [
        {"x": np.ascontiguousarray(x[b]), "w": w, "masks": masks, "ident": ident}
        for b in range(B)
    ]
    res = run_bass_kernel_spmd(nc, in_maps, core_ids=list(range(B)))
    return np.stack([res.results[b]["out"] for b in range(B)], axis=0)


# revision 10
# speedup vs baseline: 1.0516x; 1.0516x over previous
"""Trainium2 Bass kernel: batched single-head causal attention.

Problem: x [8, 2048, 1024] f32; Wq/Wk/Wv [64, 1024] f32.
  Q = x @ Wq.T; K = x @ Wk.T; V = x @ Wv.T            (per batch)
  out = softmax(mask(Q K^T / sqrt(1024))) @ V          -> [8, 2048, 64]

Sharding: data-parallel over batch B=8 across the 8 NeuronCores (one batch
element per core); the small weights are replicated.

Per-core algorithm (T=2048, C=1024, H=64, all fp32 data):
  - x is transposed on-device via PE transposes into xT [C, T] (C-chunked on
    partitions), since every projection contracts over C.
  - Projections computed in transposed layout: QT [64, T] (scaled by 1/32,
    folded into Wq on host) and a stacked KVT [128, T] (KT rows 0:64,
    VT rows 64:128).  Stacking K with V keeps KT at partition base 0 (both
    scores operands must sit in the same partition range), while VT at base
    64 is fine because it is only consumed by PE transposes.
  - V is re-transposed to natural layout V_aug [T, 65] with a ones column
    appended; the ones column makes the attention row-sums fall out of the
    same matmul that computes attn @ V.
  - Scores are computed TRANSPOSED: sT[tk, tq] = K Q^T blocks [128, 512];
    causality skips all fully-masked blocks; softmax max-subtraction is
    skipped entirely (scores are provably tiny: |s| <~ 1.5) so softmax is
    just exp + normalize; exp runs on ACT straight out of PSUM; diagonal
    blocks are multiplied by precomputed 0/1 masks (exp of masked entries
    becomes exactly 0, matching exp(-inf)).
  - outT_aug [65, 512] = V_aug^T @ expT accumulated over tk chunks: rows
    0:64 unnormalized out^T, row 64 = row-sums.  A final PE transpose gives
    natural [128, 65] chunks; DVE reciprocal + tensor_scalar multiply
    normalizes; DMA out.

Matmuls run in float32r (1 row/cycle vs 4 for float32).  The BIR verifier
requires every SBUF tensor consumed by an FP32r matmul to be produced by an
instruction with f32r output ("rounded").  All matmul operands here are
produced by ACT/DVE copies out of PSUM anyway, so those copies declare f32r
output for free; the only extra work is one rounding copy for the weights.
"""

import numpy as np

import concourse.bass as bass
import concourse.mybir as mybir
import concourse.tile as tile
from concourse import bacc
from concourse.bass_utils import run_bass_kernel_spmd

B = 8
T = 2048
C = 1024
H = 64
P = 128
NT = T // P   # 16 row chunks
NCH = C // P  # 8 contraction chunks
NB = 4        # tq blocks
BQ = 512      # tq block size
F32 = mybir.dt.float32
F32R = mybir.dt.float32r

# float32r matmuls: 1 row/cycle (vs 4 for float32) when moving free >= 256.
MM_FP32R = True


def _mm(ap):
    """Matmul-operand view."""
    return ap.bitcast(F32R) if MM_FP32R else ap


def _rnd(ap):
    """Producer-output view for tensors later consumed by f32r matmuls."""
    return ap.bitcast(F32R) if MM_FP32R else ap


def build_nc():
    nc = bacc.Bacc("TRN2", target_bir_lowering=False)
    x_d = nc.dram_tensor("x", [T, C], F32, kind="ExternalInput").ap()
    w_d = nc.dram_tensor("w", [C, 192], F32, kind="ExternalInput").ap()
    m_d = nc.dram_tensor("masks", [P, 4, BQ], F32, kind="ExternalInput").ap()
    i_d = nc.dram_tensor("ident", [P, P], F32, kind="ExternalInput").ap()
    o_d = nc.dram_tensor("out", [T, H], F32, kind="ExternalOutput").ap()

    with tile.TileContext(nc) as tc:
        _emit(nc, tc, x_d, w_d, m_d, i_d, o_d)
    nc.compile()
    return nc


def _emit(nc, tc, x_d, w_d, m_d, i_d, o_d):
    import contextlib

    ctx = contextlib.ExitStack()
    with ctx:
        consts = ctx.enter_context(tc.tile_pool(name="consts", bufs=1))
        xnat = ctx.enter_context(tc.tile_pool(name="xnat", bufs=4))
        persist = ctx.enter_context(tc.tile_pool(name="persist", bufs=1))
        expp = ctx.enter_context(tc.tile_pool(name="expp", bufs=4))
        oaugp = ctx.enter_context(tc.tile_pool(name="oaugp", bufs=2))
        outp = ctx.enter_context(tc.tile_pool(name="outp", bufs=4))
        recp = ctx.enter_context(tc.tile_pool(name="recp", bufs=4))
        # PSUM: psT 2 + psP 3 + psC 3 = 8 banks exactly
        psT = ctx.enter_context(tc.tile_pool(name="psT", bufs=2, space="PSUM"))
        psP = ctx.enter_context(tc.tile_pool(name="psP", bufs=3, space="PSUM"))
        psC = ctx.enter_context(tc.tile_pool(name="psC", bufs=3, space="PSUM"))

        # ---- constants ----
        # ident goes first on the HWDGE(sync) queue (first transposes need it);
        # w/masks ride the SWDGE(gpsimd) queue so they don't delay x intake.
        ident_sb = consts.tile([P, P], F32, tag="ident")
        nc.sync.dma_start(out=ident_sb, in_=i_d)
        w_raw = consts.tile([P, NCH, 192], F32, tag="w_raw")
        nc.gpsimd.dma_start(out=w_raw, in_=w_d.rearrange("(k p) m -> p k m", p=P))
        w_sb = consts.tile([P, NCH, 192], F32, tag="w")
        nc.vector.tensor_copy(out=_rnd(w_sb), in_=w_raw)  # f32r rounding copy
        masks_sb = consts.tile([P, 4, BQ], F32, tag="masks")
        nc.gpsimd.dma_start(out=masks_sb, in_=m_d)

        # ---- persistent tiles ----
        xT = persist.tile([P, NCH, T], F32, tag="xT")      # xT[p,k,t] = x[t, k*128+p]
        qt = persist.tile([64, T], F32, tag="qt")          # QT (pre-scaled by 1/32)
        kvt = persist.tile([P, T], F32, tag="kvt")         # rows 0:64 KT, 64:128 VT
        vaug = persist.tile([P, NT, H + 1], F32, tag="vaug")  # V chunks + ones col

        ones_sb = consts.tile([P, NT, 1], F32, tag="ones")
        nc.vector.memset(ones_sb, 1.0)
        nc.vector.tensor_copy(out=_rnd(vaug[:, :, H : H + 1]), in_=ones_sb)

        ncopy = 0  # alternate ACT / DVE for PSUM->SBUF copies

        def copy(out, in_):
            nonlocal ncopy
            if ncopy % 2 == 0:
                nc.scalar.copy(out=out, in_=in_)
            else:
                nc.vector.tensor_copy(out=out, in_=in_)
            ncopy += 1

        # lag-2 pipeline of chunk-wise score -> exp/mask -> AV matmul
        pending = []

        def flush_av(limit):
            while len(pending) > limit:
                av_t, ex_ap, i_, last_ = pending.pop(0)
                nc.tensor.matmul(
                    av_t,
                    lhsT=_mm(vaug[:, i_, 0 : H + 1]),
                    rhs=_mm(ex_ap),
                    start=(i_ == 0),
                    stop=last_,
                )

        def c_chunk(av, n, i, nchunks):
            """Scores chunk i of block n: matmul -> exp -> mask -> queue AV."""
            d = i - 4 * n
            off = 128 * d if d > 0 else 0
            sp = psC.tile([P, BQ], F32, tag="psc")
            nc.tensor.matmul(
                sp[:, off:BQ],
                lhsT=_mm(kvt[0:64, i * P : (i + 1) * P]),
                rhs=_mm(qt[:, n * BQ + off : (n + 1) * BQ]),
                start=True,
                stop=True,
            )
            ex = expp.tile([P, BQ], F32, tag="ex")
            nc.scalar.activation(
                out=_rnd(ex[:, off:BQ]),
                in_=sp[:, off:BQ],
                func=mybir.ActivationFunctionType.Exp,
            )
            if d >= 0:
                nc.vector.tensor_mul(
                    _rnd(ex[:, off:BQ]), ex[:, off:BQ], masks_sb[:, d, off:BQ]
                )
            pending.append((av[0:65, off:BQ], ex[:, off:BQ], i, i == nchunks - 1))
            flush_av(2)

        for n in range(NB):
            nchunks = 4 * (n + 1)
            # ---- stage A: load x rows (in halves), PE-transpose into xT ----
            for j in range(4 * n, 4 * n + 4):
                xt = xnat.tile([P, C], F32, tag="xt")
                for half in range(2):
                    nc.sync.dma_start(
                        out=xt[:, half * 512 : half * 512 + 512],
                        in_=x_d[j * P : (j + 1) * P, half * 512 : half * 512 + 512],
                    )
                    ps = psT.tile([P, 4, P], F32, tag="pst")
                    for kk in range(4):
                        k = half * 4 + kk
                        nc.tensor.transpose(
                            out=ps[:, kk, :],
                            in_=xt[:, k * P : (k + 1) * P],
                            identity=ident_sb,
                        )
                    copy(
                        _rnd(xT[:, half * 4 : half * 4 + 4, j * P : (j + 1) * P]),
                        ps,
                    )

            # ---- stage B(q): Q projection for tq block n ----
            q_ps = psP.tile([64, BQ], F32, tag="psp")
            for k in range(NCH):
                nc.tensor.matmul(
                    q_ps,
                    lhsT=_mm(w_sb[:, k, 0:64]),
                    rhs=_mm(xT[:, k, n * BQ : (n + 1) * BQ]),
                    start=(k == 0),
                    stop=(k == NCH - 1),
                )
            copy(_rnd(qt[:, n * BQ : (n + 1) * BQ]), q_ps)

            # ---- stage C (early chunks): depend only on OLD kvt/vaug ----
            av = psP.tile([65, BQ], F32, tag="psp")
            for i in range(4 * n):
                c_chunk(av, n, i, nchunks)

            # ---- stage B(kv): K|V projection for tq block n ----
            kv_ps = psP.tile([P, BQ], F32, tag="psp")
            for k in range(NCH):
                nc.tensor.matmul(
                    kv_ps,
                    lhsT=_mm(w_sb[:, k, 64:192]),
                    rhs=_mm(xT[:, k, n * BQ : (n + 1) * BQ]),
                    start=(k == 0),
                    stop=(k == NCH - 1),
                )
            # split copy: V half (DVE) unblocks B5 transposes; K half (ACT)
            # unblocks the diagonal score chunks. Runs in parallel.
            nc.vector.tensor_copy(
                out=_rnd(kvt[64:128, n * BQ : (n + 1) * BQ]),
                in_=kv_ps[64:128, :],
            )
            nc.scalar.copy(
                out=_rnd(kvt[0:64, n * BQ : (n + 1) * BQ]), in_=kv_ps[0:64, :]
            )

            # ---- stage B5: V natural chunks for this block ----
            for j in range(4 * n, 4 * n + 4):
                vp = psT.tile([P, H], F32, tag="pst")
                nc.tensor.transpose(
                    out=vp,
                    in_=kvt[64:128, j * P : (j + 1) * P],
                    identity=ident_sb[64:128, 64:128],
                )
                copy(_rnd(vaug[:, j, 0:H]), vp)

            # ---- stage C (diagonal chunks) ----
            for i in range(4 * n, nchunks):
                c_chunk(av, n, i, nchunks)
            flush_av(0)

            # ---- stage D: transpose back, normalize, store ----
            oa = oaugp.tile([65, BQ], F32, tag="oa")
            copy(_rnd(oa), av)
            for q in range(4):
                tq = n * 4 + q
                tp = psC.tile([P, 72], F32, tag="psc")
                nc.tensor.transpose(
                    out=tp[:, 0:65],
                    in_=oa[:, q * P : (q + 1) * P],
                    identity=ident_sb[0:65, 0:65],
                )
                r = recp.tile([P, 1], F32, tag="r")
                nc.vector.reciprocal(r, tp[:, 64:65])
                ot = outp.tile([P, H], F32, tag="ot")
                nc.vector.tensor_scalar_mul(ot, tp[:, 0:64], r)
                nc.gpsimd.dma_start(out=o_d[tq * P : (tq + 1) * P, :], in_=ot)


def host_inputs(Wq, Wk, Wv):
    """Replicated per-core constant inputs from the raw weights."""
    scale = np.float32(1.0 / np.sqrt(np.float32(C)))
    w = np.empty((C, 192), dtype=np.float32)
    w[:, 0:64] = Wq.T * scale
    w[:, 64:128] = Wk.T
    w[:, 128:192] = Wv.T
    p = np.arange(P, dtype=np.int64)[:, None, None]
    d = np.arange(4, dtype=np.int64)[None, :, None]
    j = np.arange(BQ, dtype=np.int64)[None, None, :]
    masks = ((p + 128 * d) <= j).astype(np.float32)
    ident = np.eye(P, dtype=np.float32)
    return w, masks, ident


def kernel(x, Wq, Wk, Wv):
    x = np.ascontiguousarray(np.asarray(x, dtype=np.float32))
    Wq = np.asarray(Wq, dtype=np.float32)
    Wk = np.asarray(Wk, dtype=np.float32)
    Wv = np.asarray(Wv, dtype=np.float32)
    assert x.shape == (B, T, C), x.shape

    w, masks, ident = host_inputs(Wq, Wk, Wv)
    nc = build_nc()
    in_maps = [
        {"x": np.ascontiguousarray(x[b]), "w": w, "masks": masks, "ident": ident}
        for b in range(B)
    ]
    res = run_bass_kernel_spmd(nc, in_maps, core_ids=list(range(B)))
    return np.stack([res.results[b]["out"] for b in range(B)], axis=0)
